# revision 4
# baseline (speedup 1.0000x reference)
"""DifferentialAttention (B=2, S=2048, D=2048, H=16, KVH=8) on 8 TRN2 NeuronCores.

Sharding: 8 cores = 2 (batch) x 4 (tensor-parallel head groups).
Core c = 4*b + r handles batch b and real heads 4r..4r+3:
  - column-parallel q/k/v projections (q heads 8r..8r+7, k heads 4r..4r+3,
    v heads 2r..2r+1), full causal differential attention for those heads,
  - row-parallel partial o_proj; host sums the 4 partials per batch.

Device math (per core), all fp32 data with float32r matmuls:
  - host passes x^T and W^T slices so every matmul contracts on partitions
  - scores computed transposed  S^T[k,q] = k . q  so exp -> AV needs no
    on-chip transposes
  - softmax without max subtraction (scores ~ N(0,1)); row sums and their
    partition broadcast fused into one all-ones matmul
  - RMS-norm folded:  out = u * rsqrt(mean_d(u^2) + eps*r1^2)  with
    u = O1 - (lam*r1/r2)*O2,  O = E@v unnormalized,  r = rowsum(E);
    subln weight and (1 - lambda_init) folded into Wo on the host
  - causal masking: matmuls sliced to valid columns; 0/1 triangular mask
    multiplied into the single mixed 128x128 block per diagonal tile
"""

import math
import numpy as np

B, S, D = 2, 2048, 2048
H, KVH = 16, 8
Dh = 64
TP = 4
NCORES = 8
LAYER_IDX = 2
LAMBDA_INIT = 0.8 - 0.6 * math.exp(-0.3 * LAYER_IDX)
EPS = 1e-5
ROPE_THETA = 10000.0

_CACHE = {}


def _build_nc():
    import concourse.bass as bass  # noqa: F401
    import concourse.tile as tile
    from concourse import bacc, mybir

    F32 = mybir.dt.float32
    F32R = mybir.dt.float32r
    Act = mybir.ActivationFunctionType
    Alu = mybir.AluOpType

    nc = bacc.Bacc("TRN2", target_bir_lowering=False, debug=False)

    xT = nc.dram_tensor("xT", [D, S], F32R, kind="ExternalInput")
    wqT = nc.dram_tensor("wqT", [D, 512], F32R, kind="ExternalInput")
    wkT = nc.dram_tensor("wkT", [D, 256], F32R, kind="ExternalInput")
    wvT = nc.dram_tensor("wvT", [D, 256], F32R, kind="ExternalInput")
    woT = nc.dram_tensor("woT", [512, D], F32R, kind="ExternalInput")
    cosT_d = nc.dram_tensor("cosT", [128, S], F32, kind="ExternalInput")
    ssinT_d = nc.dram_tensor("ssinT", [128, S], F32, kind="ExternalInput")
    tri_d = nc.dram_tensor("tri", [128, 128], F32R, kind="ExternalInput")
    ones_d = nc.dram_tensor("ones", [128, 128], F32R, kind="ExternalInput")
    lam_d = nc.dram_tensor("lam", [128, 1], F32, kind="ExternalInput")
    out_d = nc.dram_tensor("out", [S, D], F32, kind="ExternalOutput")

    KD = D // 128  # 16 contraction tiles

    with tile.TileContext(nc) as tc:
        with tc.tile_pool(name="const", bufs=1) as constp, \
             tc.tile_pool(name="persist", bufs=1) as persist:

            cosT = constp.tile([128, S], F32, tag="cos")
            ssinT = constp.tile([128, S], F32, tag="ssin")
            tri = constp.tile([128, 128], F32R, tag="tri")
            ones = constp.tile([128, 128], F32R, tag="ones")
            lam = constp.tile([128, 1], F32, tag="lam")
            nc.sync.dma_start(out=cosT[:], in_=cosT_d[:])
            nc.sync.dma_start(out=ssinT[:], in_=ssinT_d[:])
            nc.sync.dma_start(out=tri[:], in_=tri_d[:])
            nc.sync.dma_start(out=ones[:], in_=ones_d[:])
            nc.sync.dma_start(out=lam[:], in_=lam_d[:])

            # persistent intermediates
            qT_sb = [persist.tile([128, S], F32R, tag=f"qT{m}", name=f"qT{m}") for m in range(4)]
            kTd = [persist.tile([128, S], F32R, tag=f"kTd{h}", name=f"kTd{h}") for h in range(4)]
            v_sb = [persist.tile([128, 256], F32R, tag=f"v{ms}", name=f"v{ms}") for ms in range(16)]
            otf = [persist.tile([128, S], F32R, tag=f"otf{p}", name=f"otf{p}") for p in range(4)]

            # ---------------- Phase A: projections + RoPE ----------------
            with tc.tile_pool(name="xtp", bufs=16) as xtp, \
                 tc.tile_pool(name="wstream", bufs=2) as wsp, \
                 tc.tile_pool(name="ropet", bufs=1) as rp, \
                 tc.tile_pool(name="psA", bufs=8, space="PSUM") as psA:

                def rope_core(ps, gc0):
                    """RoPE on a [128, 512] psum tile; returns (ra, rt) to add."""
                    gsl = slice(gc0, gc0 + 512)
                    qraw = rp.tile([128, 512], F32, tag="qraw", name="qraw")
                    nc.scalar.copy(qraw[:], ps[:])
                    qsw = rp.tile([128, 512], F32, tag="qsw", name="qsw")
                    for blk in range(4):
                        sb_ = (blk ^ 1) * 32
                        nc.sync.dma_start(
                            out=qsw[blk * 32:blk * 32 + 32, :],
                            in_=qraw[sb_:sb_ + 32, :],
                        )
                    ra = rp.tile([128, 512], F32, tag="ra", name="ra")
                    nc.vector.tensor_mul(ra[:], qraw[:], cosT[:, gsl])
                    rt = rp.tile([128, 512], F32, tag="rt", name="rt")
                    nc.vector.tensor_mul(rt[:], qsw[:], ssinT[:, gsl])
                    return ra, rt

                def rope_epilogue(ps, dst, gc0):
                    ra, rt = rope_core(ps, gc0)
                    nc.vector.tensor_add(dst[:, gc0:gc0 + 512], ra[:], rt[:])

                def rope_epilogue_kdup(ps, m, gc0):
                    """RoPE then duplicate each 64-row head half into kTd[2m+e]."""
                    gsl = slice(gc0, gc0 + 512)
                    ra, rt = rope_core(ps, gc0)
                    ktmp = rp.tile([128, 512], F32R, tag="ktmp", name="ktmp")
                    nc.vector.tensor_add(ktmp[:], ra[:], rt[:])
                    for e in range(2):
                        src = ktmp[e * 64:e * 64 + 64, :]
                        nc.sync.dma_start(out=kTd[2 * m + e][0:64, gsl], in_=src)
                        nc.sync.dma_start(out=kTd[2 * m + e][64:128, gsl], in_=src)

                for sh in range(4):
                    c0 = 512 * sh
                    xt = []
                    for kd in range(KD):
                        t = xtp.tile([128, 512], F32R, tag="xt", name=f"xt{kd}")
                        nc.sync.dma_start(
                            out=t[:], in_=xT[kd * 128:kd * 128 + 128, c0:c0 + 512]
                        )
                        xt.append(t)

                    # q projection: qT[j, s] for j in [0,512)
                    qps = {}
                    for kd in range(KD):
                        wqb = wsp.tile([128, 512], F32R, tag="wq")
                        nc.sync.dma_start(out=wqb[:], in_=wqT[kd * 128:kd * 128 + 128, :])
                        for m in range(4):
                            if kd == 0:
                                qps[m] = psA.tile([128, 512], F32, tag="pa", name=f"qps{m}")
                            nc.tensor.matmul(
                                qps[m][:],
                                wqb[:, m * 128:m * 128 + 128],
                                xt[kd][:],
                                start=(kd == 0), stop=(kd == KD - 1),
                            )
                    for m in range(4):
                        rope_epilogue(qps[m], qT_sb[m], c0)

                    # k projection
                    kps = {}
                    for kd in range(KD):
                        wkb = wsp.tile([128, 256], F32R, tag="wk")
                        nc.sync.dma_start(out=wkb[:], in_=wkT[kd * 128:kd * 128 + 128, :])
                        for m in range(2):
                            if kd == 0:
                                kps[m] = psA.tile([128, 512], F32, tag="pa", name=f"kps{m}")
                            nc.tensor.matmul(
                                kps[m][:],
                                wkb[:, m * 128:m * 128 + 128],
                                xt[kd][:],
                                start=(kd == 0), stop=(kd == KD - 1),
                            )
                    for m in range(2):
                        rope_epilogue_kdup(kps[m], m, c0)

                    # v projection: v[s, j] (s on partitions)
                    vps = {}
                    for kd in range(KD):
                        wvb = wsp.tile([128, 256], F32R, tag="wv")
                        nc.sync.dma_start(out=wvb[:], in_=wvT[kd * 128:kd * 128 + 128, :])
                        for ms in range(4):
                            if kd == 0:
                                vps[ms] = psA.tile([128, 512], F32, tag="pa", name=f"vps{ms}")
                            nc.tensor.matmul(
                                vps[ms][:, 0:256],
                                xt[kd][:, ms * 128:ms * 128 + 128],
                                wvb[:],
                                start=(kd == 0), stop=(kd == KD - 1),
                            )
                    for ms in range(4):
                        nc.scalar.copy(v_sb[sh * 4 + ms][:], vps[ms][:, 0:256])

            # ---------------- Phase B: attention ----------------
            with tc.tile_pool(name="etp", bufs=3) as etp, \
                 tc.tile_pool(name="ebp", bufs=1) as ebp, \
                 tc.tile_pool(name="psAcc", bufs=4, space="PSUM") as psAcc, \
                 tc.tile_pool(name="psS", bufs=4, space="PSUM") as psS:

                for p in range(4):
                    vh = p // 2
                    for qi in range(4):
                        q0 = 512 * qi
                        nki = 4 * qi + 4
                        OT1 = psAcc.tile([128, 512], F32, tag="acc")
                        OT2 = psAcc.tile([128, 512], F32, tag="acc")
                        R1 = psAcc.tile([128, 512], F32, tag="acc")
                        R2 = psAcc.tile([128, 512], F32, tag="acc")
                        for ki in range(nki):
                            j = ki - 4 * qi
                            vc = 128 * j if j > 0 else 0
                            ksl = slice(ki * 128, ki * 128 + 128)
                            st, sp_ = (ki == 0), (ki == nki - 1)
                            S1 = psS.tile([128, 512], F32, tag="s")
                            S2 = psS.tile([128, 512], F32, tag="s")
                            nc.tensor.matmul(
                                S1[:, vc:512],
                                kTd[p][0:64, ksl],
                                qT_sb[p][0:64, q0 + vc:q0 + 512],
                                start=True, stop=True,
                            )
                            nc.tensor.matmul(
                                S2[:, vc:512],
                                kTd[p][64:128, ksl],
                                qT_sb[p][64:128, q0 + vc:q0 + 512],
                                start=True, stop=True,
                            )
                            ET1 = etp.tile([128, 512], F32R, tag="e1")
                            ET2 = etp.tile([128, 512], F32R, tag="e2")
                            nc.scalar.activation(ET1[:, vc:512], S1[:, vc:512], Act.Exp)
                            nc.scalar.activation(ET2[:, vc:512], S2[:, vc:512], Act.Exp)
                            if j >= 0:  # diagonal tile: mask mixed 128x128 block
                                msl = slice(vc, vc + 128)
                                nc.vector.tensor_mul(ET1[:, msl], ET1[:, msl], tri[:])
                                nc.vector.tensor_mul(ET2[:, msl], ET2[:, msl], tri[:])
                            vt = v_sb[ki][:, vh * 128:vh * 128 + 128]
                            nc.tensor.matmul(OT1[:, vc:512], vt, ET1[:, vc:512],
                                             start=st, stop=sp_)
                            nc.tensor.matmul(R1[:, vc:512], ones[:], ET1[:, vc:512],
                                             start=st, stop=sp_)
                            nc.tensor.matmul(OT2[:, vc:512], vt, ET2[:, vc:512],
                                             start=st, stop=sp_)
                            nc.tensor.matmul(R2[:, vc:512], ones[:], ET2[:, vc:512],
                                             start=st, stop=sp_)

                        # epilogue: normalize + differential combine + RMS
                        rcp2 = ebp.tile([128, 512], F32, tag="rcp2")
                        nc.vector.reciprocal(rcp2[:], R2[:])
                        mb = ebp.tile([128, 512], F32, tag="mb")
                        nc.vector.scalar_tensor_tensor(
                            mb[:], R1[:], lam[:, 0:1], rcp2[:], Alu.mult, Alu.mult)
                        tt = ebp.tile([128, 512], F32, tag="tt")
                        nc.vector.tensor_mul(tt[:], OT2[:], mb[:])
                        u = ebp.tile([128, 512], F32, tag="u")
                        nc.vector.tensor_sub(u[:], OT1[:], tt[:])
                        sq = ebp.tile([128, 512], F32R, tag="sq")
                        nc.scalar.square(sq[:], u[:])
                        varp = psS.tile([128, 512], F32, tag="s")
                        nc.tensor.matmul(varp[:], ones[:], sq[:], start=True, stop=True)
                        r1e = ebp.tile([128, 512], F32, tag="r1e")
                        nc.vector.tensor_scalar_mul(r1e[:], R1[:], math.sqrt(EPS))
                        t2 = ebp.tile([128, 512], F32, tag="t2")
                        nc.vector.tensor_mul(t2[:], r1e[:], r1e[:])
                        pre = ebp.tile([128, 512], F32, tag="pre")
                        nc.vector.scalar_tensor_tensor(
                            pre[:], varp[:], 1.0 / 128.0, t2[:], Alu.mult, Alu.add)
                        sp2 = ebp.tile([128, 512], F32, tag="sp")
                        nc.scalar.sqrt(sp2[:], pre[:])
                        sf = ebp.tile([128, 512], F32, tag="sf")
                        nc.vector.reciprocal(sf[:], sp2[:])
                        nc.vector.tensor_mul(otf[p][:, q0:q0 + 512], u[:], sf[:])

            # ---------------- Phase C: o_proj (row-parallel partial) -------
            with tc.tile_pool(name="wop", bufs=2) as wop, \
                 tc.tile_pool(name="outp", bufs=3) as outp, \
                 tc.tile_pool(name="psC", bufs=4, space="PSUM") as psC:
                for n in range(4):
                    wo_t = []
                    for kc in range(4):
                        t = wop.tile([128, 512], F32R, tag=f"wo{kc}", name=f"wo{kc}")
                        nc.sync.dma_start(
                            out=t[:],
                            in_=woT[kc * 128:kc * 128 + 128, n * 512:n * 512 + 512],
                        )
                        wo_t.append(t)
                    for m in range(16):
                        ps = psC.tile([128, 512], F32, tag="pc")
                        for kc in range(4):
                            nc.tensor.matmul(
                                ps[:],
                                otf[kc][:, m * 128:m * 128 + 128],
                                wo_t[kc][:],
                                start=(kc == 0), stop=(kc == 3),
                            )
                        osb = outp.tile([128, 512], F32, tag="ob")
                        nc.scalar.copy(osb[:], ps[:])
                        nc.sync.dma_start(
                            out=out_d[m * 128:m * 128 + 128, n * 512:n * 512 + 512],
                            in_=osb[:],
                        )

    nc.compile()
    return nc


def _host_tables():
    inv = ROPE_THETA ** (-np.arange(Dh, dtype=np.float64) / Dh)
    pos = np.arange(S, dtype=np.float64)
    fr = pos[:, None] * inv[None, :]              # [S, 64]
    cos = np.cos(fr).astype(np.float32)           # [S, 64]
    sin = np.sin(fr).astype(np.float32)
    d = np.arange(128) % 64
    cosT = cos[:, d].T.copy()                     # [128, S]
    sgn = np.where((np.arange(128) % 64) < 32, -1.0, 1.0).astype(np.float32)
    ssinT = (sin[:, d].T * sgn[:, None]).copy()
    tri = np.triu(np.ones((128, 128), np.float32))  # tri[k, q] = 1 if q >= k
    ones = np.ones((128, 128), np.float32)
    return np.ascontiguousarray(cosT), np.ascontiguousarray(ssinT), tri, ones


def kernel(hidden_states, Wq, Wk, Wv, Wo,
           lambda_q1, lambda_k1, lambda_q2, lambda_k2, subln_weight):
    from concourse.bass_utils import run_bass_kernel_spmd

    if "nc" not in _CACHE:
        _CACHE["nc"] = _build_nc()
        _CACHE["tables"] = _host_tables()
    nc = _CACHE["nc"]
    cosT, ssinT, tri, ones = _CACHE["tables"]

    f32 = np.float32
    hs = np.asarray(hidden_states, f32)
    Wq = np.asarray(Wq, f32)
    Wk = np.asarray(Wk, f32)
    Wv = np.asarray(Wv, f32)
    Wo = np.asarray(Wo, f32)
    subln = np.asarray(subln_weight, f32)

    lam1 = np.exp(np.sum(np.asarray(lambda_q1, f32) * np.asarray(lambda_k1, f32),
                         dtype=f32))
    lam2 = np.exp(np.sum(np.asarray(lambda_q2, f32) * np.asarray(lambda_k2, f32),
                         dtype=f32))
    lam_full = f32(lam1 - lam2 + LAMBDA_INIT)
    lam_arr = np.full((128, 1), lam_full, f32)

    scale = f32(Dh ** -0.5)
    wprime = (np.tile(subln, H) * f32(1.0 - LAMBDA_INIT)).astype(f32)  # [2048]
    WoS = Wo * wprime[None, :]

    in_maps = []
    for c in range(NCORES):
        b, r = c // TP, c % TP
        in_maps.append({
            "xT": np.ascontiguousarray(hs[b].T),
            "wqT": np.ascontiguousarray((Wq[512 * r:512 * r + 512, :] * scale).T),
            "wkT": np.ascontiguousarray(Wk[256 * r:256 * r + 256, :].T),
            "wvT": np.ascontiguousarray(Wv[256 * r:256 * r + 256, :].T),
            "woT": np.ascontiguousarray(WoS[:, 512 * r:512 * r + 512].T),
            "cosT": cosT, "ssinT": ssinT, "tri": tri, "ones": ones,
            "lam": lam_arr,
        })

    res = run_bass_kernel_spmd(nc, in_maps, core_ids=list(range(NCORES)))
    out = np.zeros((B, S, D), f32)
    for c in range(NCORES):
        out[c // TP] += res.results[c]["out"]
    return out
